# revision 48
# baseline (speedup 1.0000x reference)
"""DSS (Diagonal State Space) layer as a Bass/Tile kernel for 8 Trainium2 NeuronCores.

Per core (channels H sharded 8 x 128), overlap-save FFT convolution with
chunk-reused half-transforms:
  1. DSS-exp kernel k[l,h] = Re(sum_n W[h,n] z^l), z = exp(dt_h*Lambda_n), built
     on-device: transcendentals via fused scalar_tensor_tensor Horner chains
     (ACT LUTs are too coarse for the z^511 power compounding), then complex
     doubling chains build W z^b (b<32, fp32) and z^(32a) (a<16) planes, and a
     per-channel pair of contraction-64 PE matmuls does the mode-sum.
  2. K_f = half-rfft of k via PE matmuls reusing the forward DFT stationaries
     (bf16 moving side: full PE rate at 128-wide free dim); skip connection
     u*D folds into the filter as K' = K + D.
  3. Overlap-save, F=1024, hop 512: block b's spectrum is X_b = G_{b-1} +
     (-1)^f G_b, where G_c is the half-transform of 512-sample chunk c,
     computed ONCE per chunk -- halving forward matmul work. Frequencies are
     reordered even/odd on the host so the (-1)^f combine is a plain per-tile
     add (even tiles) / subtract (odd tiles) in bf16 on DVE (2x rate).
  4. Pointwise complex multiply in bf16 split across DVE and gpsimd; inverse
     rfft as PE matmuls (bf16 moving, ft-major accumulation so it starts after
     the first frequency tile's pointwise).

Pipeline: u input (bf16) streams on the SP queue; six chunk-group forward
transforms are emitted up front so the PE covers the whole prologue chain;
combines run one block ahead and pointwise one block ahead of the inverse;
evacuations and y output ride the ACT queue. All engine ops keep partition
base 0 and matched fp32-family operand dtypes (BIR verifier requirements).
"""

import sys

for _p in ("/opt/trn_rl_repo", "/opt/trn_rl_repo/concourse"):
    if _p not in sys.path:
        sys.path.insert(0, _p)

import numpy as np
import ml_dtypes
from contextlib import ExitStack

import concourse.bacc as bacc
import concourse.tile as tile
import concourse.mybir as mybir

dt = mybir.dt
f32 = np.float32

B, L, H, N = 4, 4096, 1024, 64
LK = 512
F = 1024          # FFT length (overlap-save)
HOP = 512         # block hop = new samples per block
NCORES = 8
HS = H // NCORES  # 128 channels per core
NBLK = L // HOP   # 8
NFT = 4           # freq tiles of 128 (perm order: ft0,1 = even freqs, ft2,3 = odd)
NJ = 4            # contraction chunks (128 each) per 512-sample half-transform
NLT = 4           # output l-tiles per block


# ---------------------------------------------------------------- host constants
def build_constants():
    evens = np.arange(0, 512, 2)
    odds = np.arange(1, 512, 2)
    perm = np.concatenate([evens, odds])        # packed-freq reordering

    l5 = np.arange(512, dtype=np.float64)[:, None]
    fr = perm[None, :].astype(np.float64)
    ang = 2 * np.pi * l5 * fr / F
    C = np.cos(ang)                             # [512 l, 512 perm-freq]
    S = -np.sin(ang)
    S[:, 0] = (-1.0) ** np.arange(512)          # Nyquist row packed into sin slot 0

    CH = np.zeros((2, NFT, NJ, 128, 128))
    for t_ in range(2):
        M = C if t_ == 0 else S
        for ft in range(NFT):
            for j in range(NJ):
                CH[t_, ft, j] = M[128 * j:128 * j + 128, 128 * ft:128 * ft + 128]

    lc = 512 + np.arange(512, dtype=np.float64)[None, :]   # valid circular outputs
    frc = perm[:, None].astype(np.float64)
    cf_ = np.where(frc == 0, 1.0, 2.0)
    Ar = cf_ * np.cos(2 * np.pi * frc * lc / F) / F
    Ai = -(2.0 / F) * np.sin(2 * np.pi * frc * lc / F)
    Ai[0, :] = ((-1.0) ** lc[0]) / F                        # Nyquist inverse row
    AI = np.zeros((2, NFT, NLT, 128, 128))
    for ft in range(NFT):
        for lt in range(NLT):
            AI[0, ft, lt] = Ar[128 * ft:128 * ft + 128, 128 * lt:128 * lt + 128]
            AI[1, ft, lt] = Ai[128 * ft:128 * ft + 128, 128 * lt:128 * lt + 128]
    return CH.astype(ml_dtypes.bfloat16), AI.astype(ml_dtypes.bfloat16)


# Horner coefficient lists (highest degree first)
def _fact(k):
    r = 1.0
    for i in range(2, k + 1):
        r *= i
    return r


EXP7 = [1.0 / _fact(k) for k in range(7, -1, -1)]            # e^x, |x| <~ 0.35
EXP8 = [1.0 / _fact(k) for k in range(8, -1, -1)]            # e^x, |x| <~ 1.0
SIN9 = [1.0 / _fact(9), -1.0 / _fact(7), 1.0 / _fact(5), -1.0 / _fact(3), 1.0]
COSC = [1.0 / _fact(10), -1.0 / _fact(8), 1.0 / _fact(6), -1.0 / _fact(4),
        1.0 / _fact(2)]


class _Prog:
    def __init__(self):
        self.nc = None
        self.built = False


_prog = _Prog()


def _emit_kernel(nc, tc, ctx, aps):
    V = nc.vector
    A = nc.scalar
    T = nc.tensor
    u_ap = aps["u"]; y_ap = aps["y"]
    ch_ap = aps["CH"]; ai_ap = aps["AI"]
    TT = V.tensor_tensor
    GT = nc.gpsimd.tensor_tensor
    op = mybir.AluOpType

    # ---------------- pools
    p_ch = ctx.enter_context(tc.tile_pool(name="ch", bufs=1))
    p_ai = ctx.enter_context(tc.tile_pool(name="ai", bufs=1))
    p_uch = ctx.enter_context(tc.tile_pool(name="uch", bufs=10))
    p_g = ctx.enter_context(tc.tile_pool(name="g", bufs=6))
    p_x = ctx.enter_context(tc.tile_pool(name="x", bufs=2))
    p_tmp = ctx.enter_context(tc.tile_pool(name="tmp", bufs=3))
    p_yf = ctx.enter_context(tc.tile_pool(name="yf", bufs=2))
    p_yout = ctx.enter_context(tc.tile_pool(name="yout", bufs=2))
    p_krep = ctx.enter_context(tc.tile_pool(name="krep", bufs=1))
    p_kc = ctx.enter_context(tc.tile_pool(name="kc", bufs=1))
    p_ks = ctx.enter_context(tc.tile_pool(name="ks", bufs=4))
    p_gw = ctx.enter_context(tc.tile_pool(name="gw", bufs=1))
    p_zp = ctx.enter_context(tc.tile_pool(name="zp", bufs=9))
    p_small = ctx.enter_context(tc.tile_pool(name="small", bufs=1))
    p_gwtmp = ctx.enter_context(tc.tile_pool(name="gwtmp", bufs=1))
    p_drep = ctx.enter_context(tc.tile_pool(name="drep", bufs=1))
    p_ps = ctx.enter_context(tc.tile_pool(name="ps", bufs=6, space="PSUM"))
    p_psk = ctx.enter_context(tc.tile_pool(name="psk", bufs=2, space="PSUM"))

    # ---------------- small parameter tiles first (they gate the whole k prologue)
    logdt = p_small.tile([N, HS], dt.float32, tag="logdt")
    A.dma_start(logdt[:], aps["logdt"][:].broadcast_to([N, HS]))
    Lre_c = p_small.tile([N, 1], dt.float32, tag="lre")
    A.dma_start(Lre_c[:], aps["Lre"][:].transpose([1, 0]))
    Lim_c = p_small.tile([N, 1], dt.float32, tag="lim")
    A.dma_start(Lim_c[:], aps["Lim"][:].transpose([1, 0]))
    dtile = p_small.tile([1, HS], dt.float32, tag="dtile")
    A.dma_start(dtile[:], aps["D"][:])
    wret = p_small.tile([N, HS], dt.float32, tag="wret")
    A.dma_start(wret[:], aps["Wre"][:])
    wimt = p_small.tile([N, HS], dt.float32, tag="wimt")
    A.dma_start(wimt[:], aps["Wim"][:])

    # ---------------- stationaries: CH on SP queue (gates forward), AI on ACT queue
    ch_big = {}

    def load_ch(ft):
        for t_ in range(2):
            tl = p_ch.tile([128, NJ, 128], dt.bfloat16, tag=f"ch{t_}_{ft}",
                           name=f"chb{t_}_{ft}")
            eng = nc.sync if t_ == 0 else A
            eng.dma_start(tl[:], ch_ap[t_, ft].transpose([1, 0, 2]))
            ch_big[(t_, ft)] = tl

    load_ch(0)


    def ch_tile(j, t_, ft):
        return ch_big[(t_, ft)][:, j, :]

    def ai_tile(t_, ft, lt):
        return ai_big[(t_, ft)][:, lt, :]

    # u chunks: all on the SP queue (no cross-dependencies -> never head-of-line blocked)
    chunks = {}

    def get_chunk(c):
        if c not in chunks:
            t_u = p_uch.tile([128, 4, 128], dt.bfloat16, tag="uch", name=f"uch{c}")
            nc.sync.dma_start(t_u[:], u_ap[:, 128 * c:128 * c + 128, :].transpose([1, 0, 2]))
            chunks[c] = t_u
        return chunks[c]

    for c in range(4):
        get_chunk(c)
    for ft in range(1, NFT):
        load_ch(ft)
    for c in range(4, 12):
        get_chunk(c)

    ai_big = {}
    for t_ in range(2):
        for ft in range(NFT):
            tl = p_ai.tile([128, NLT, 128], dt.bfloat16, tag=f"ai{t_}_{ft}",
                           name=f"aib{t_}_{ft}")
            nc.sync.dma_start(tl[:], ai_ap[t_, ft].transpose([1, 0, 2]))
            ai_big[(t_, ft)] = tl

    # zero G planes for the block -1 "previous" spectrum
    zg = p_small.tile([128, 512], dt.bfloat16, tag="zg")
    V.memset(zg[:], 0.0)

    # ---------------- half-transform G of one 512-sample chunk-group
    gplanes = {}

    def emit_G(b):
        planes = []
        for ft in range(NFT):
            pc = p_ps.tile([128, 512], dt.float32, tag="ps", name=f"gpc{b}_{ft}")
            psn = p_ps.tile([128, 512], dt.float32, tag="ps", name=f"gps{b}_{ft}")
            for j in range(NJ):
                ch = get_chunk(4 * b + j)
                mv = ch[:].rearrange("p b h -> p (b h)")
                T.matmul(pc[:], ch_tile(j, 0, ft), mv, start=(j == 0), stop=(j == NJ - 1))
                T.matmul(psn[:], ch_tile(j, 1, ft), mv, start=(j == 0), stop=(j == NJ - 1))
            gc = p_g.tile([128, 512], dt.bfloat16, tag=f"g{ft}c", bufs=6, name=f"g{b}_{ft}c")
            A.copy(gc[:], pc[:])
            gs = p_g.tile([128, 512], dt.bfloat16, tag=f"g{ft}s", bufs=6, name=f"g{b}_{ft}s")
            A.copy(gs[:], psn[:])
            planes.append((gc, gs))
        gplanes[b] = planes

    # ---------------- prologue part 1: transcendentals + power chains.
    # All PE pieces here (outer products, D_rep) are emitted BEFORE the G
    # matmuls so the DVE chain is never gated on the forward-transform queue.
    STT = V.scalar_tensor_tensor
    GP = nc.gpsimd

    def horner(dst, x, coefs, eng):
        # P(x) = sum c_k x^k via fused steps s <- (s + c)*x, one op per coeff
        eng.tensor_scalar_mul(dst, x, float(coefs[0]))
        for c in coefs[1:-1]:
            eng.scalar_tensor_tensor(dst, dst, float(c), x, op.add, op.mult)
        eng.tensor_scalar_add(dst, dst, float(coefs[-1]))

    # dt = exp(logdt) = (exp(logdt/8))^8, computed on the 64-partition
    # broadcast copy so the n x h outer products become per-partition-scalar
    # DVE ops (no PE in the chain's critical path)
    x8 = p_small.tile([N, HS], dt.float32, tag="x8")
    V.tensor_scalar_mul(x8[:], logdt[:], 0.125)
    e8 = p_small.tile([N, HS], dt.float32, tag="e8")
    horner(e8[:], x8[:], EXP7, V)
    dtv = p_small.tile([N, HS], dt.float32, tag="dtv")
    t_a = p_small.tile([N, HS], dt.float32, tag="sq1")
    TT(t_a[:], e8[:], e8[:], op.mult)
    t_b = p_small.tile([N, HS], dt.float32, tag="sq2")
    TT(t_b[:], t_a[:], t_a[:], op.mult)
    TT(dtv[:], t_b[:], t_b[:], op.mult)

    # -exp(Lre) = -(exp(Lre/8))^8 on [N, 1] columns
    xl = p_small.tile([N, 1], dt.float32, tag="xl")
    V.tensor_scalar_mul(xl[:], Lre_c[:], 0.125)
    el8 = p_small.tile([N, 1], dt.float32, tag="el8")
    horner(el8[:], xl[:], EXP7, V)
    t_c = p_small.tile([N, 1], dt.float32, tag="sq3")
    TT(t_c[:], el8[:], el8[:], op.mult)
    t_d = p_small.tile([N, 1], dt.float32, tag="sq4")
    TT(t_d[:], t_c[:], t_c[:], op.mult)
    negel = p_small.tile([N, 1], dt.float32, tag="negel")
    t_e = p_small.tile([N, 1], dt.float32, tag="sq5")
    TT(t_e[:], t_d[:], t_d[:], op.mult)
    V.tensor_scalar_mul(negel[:], t_e[:], -1.0)

    # half-angle pieces: a[n,h] = -e^{Lre_n} dt_h ; b[n,h] = Lim_n dt_h
    ah = p_small.tile([N, HS], dt.float32, tag="ah")
    V.tensor_scalar(ah[:], dtv[:], negel[:], 0.5, op.mult, op.mult)
    bh = p_small.tile([N, HS], dt.float32, tag="bh")
    V.tensor_scalar(bh[:], dtv[:], Lim_c[:], 0.5, op.mult, op.mult)

    # trig: exp on DVE, cos-poly on gpsimd, sin-poly on DVE (concurrent streams)
    ea = p_small.tile([N, HS], dt.float32, tag="ea")
    horner(ea[:], ah[:], EXP8, V)
    ub = p_small.tile([N, HS], dt.float32, tag="ub")
    TT(ub[:], bh[:], bh[:], op.mult)
    cp = p_small.tile([N, HS], dt.float32, tag="cp")
    horner(cp[:], ub[:], COSC, V)
    cb = p_small.tile([N, HS], dt.float32, tag="cb")
    tq = p_small.tile([N, HS], dt.float32, tag="hornq", bufs=2)
    GT(tq[:], cp[:], ub[:], op.mult)
    V.tensor_scalar(cb[:], tq[:], -1.0, 1.0, op.mult, op.add)   # cos = 1 - u*P(u)
    sp = p_small.tile([N, HS], dt.float32, tag="sp")
    horner(sp[:], ub[:], SIN9, V)
    sb = p_small.tile([N, HS], dt.float32, tag="sb")
    TT(sb[:], sp[:], bh[:], op.mult)          # sin(b/2)

    wre = p_small.tile([N, HS], dt.float32, tag="wre")
    TT(wre[:], ea[:], cb[:], op.mult)
    wim = p_small.tile([N, HS], dt.float32, tag="wim")
    TT(wim[:], ea[:], sb[:], op.mult)

    def csq_parts(dre, dim_, sre, sim):
        t2 = p_small.tile([N, HS], dt.float32, tag="csq2", bufs=2)
        GT(t2[:], sim, sim, op.mult)
        t1 = p_small.tile([N, HS], dt.float32, tag="csq1", bufs=2)
        TT(t1[:], sre, sre, op.mult)
        TT(dre, t1[:], t2[:], op.subtract)
        STT(dim_, sre, 2.0, sim, op.mult, op.mult)   # 2*sre*sim fused

    def new_zpair(nm):
        zr = p_zp.tile([N, HS], dt.float32, tag="zp", name=f"{nm}r")
        zi = p_zp.tile([N, HS], dt.float32, tag="zp", name=f"{nm}i")
        return zr, zi

    # ---------------- GW planes [N, 32, HS] b-major: Re(W z^b) and -Im(W z^b)
    # in separate base-0 tiles; Z [N, 16, HS]: Re / +Im of z^(32a). All chain
    # ops stay partition-aligned at base 0; the mode-sum contracts re and im
    # with two accumulating matmuls per channel.
    GWre_t = p_gw.tile([N, 32, HS], dt.float32r, tag="gwre")
    GWim_t = p_gw.tile([N, 32, HS], dt.float32r, tag="gwim")
    Zre_t = p_gw.tile([N, 16, HS], dt.float32r, tag="zre")
    Zim_t = p_gw.tile([N, 16, HS], dt.float32r, tag="zim")
    GWre = GWre_t[:]
    GWim = GWim_t[:]
    Zre = Zre_t[:]
    Zim = Zim_t[:]
    V.tensor_scalar_mul(GWre[:, 0, :], wret[:], 1.0)
    V.tensor_scalar_mul(GWim[:, 0, :], wimt[:], -1.0)

    def cdouble_seg(pre, pim, zr, zi, s0, d0, w, conj_stored):
        # planes[d0:d0+w, :] = planes[s0:s0+w, :] * z (b-major); DVE gets
        # pre-mults + cross-re + add-re, gpsimd gets cross-im + add-im
        zre = zr[:].unsqueeze(1).broadcast_to([N, w, HS])
        zim = zi[:].unsqueeze(1).broadcast_to([N, w, HS])
        t2 = p_gwtmp.tile([N, 8, HS], dt.float32, tag="gt2", bufs=2)
        t4 = p_gwtmp.tile([N, 8, HS], dt.float32, tag="gt2", bufs=2)
        GT(t4[:, 0:w, :], pre[:, s0:s0 + w, :], zim, op.mult)
        TT(t2[:, 0:w, :], pim[:, s0:s0 + w, :], zim, op.mult)
        TT(pre[:, d0:d0 + w, :], pre[:, s0:s0 + w, :], zre, op.mult)
        TT(pim[:, d0:d0 + w, :], pim[:, s0:s0 + w, :], zre, op.mult)
        TT(pre[:, d0:d0 + w, :], pre[:, d0:d0 + w, :], t2[:, 0:w, :],
           op.add if conj_stored else op.subtract)
        GP.tensor_tensor(pim[:, d0:d0 + w, :], pim[:, d0:d0 + w, :], t4[:, 0:w, :],
                         op.subtract if conj_stored else op.add)

    def cdouble(pre, pim, zr, zi, w, conj_stored):
        cdouble_seg(pre, pim, zr, zi, 0, w, w, conj_stored)

    # a=0 plane of Z is the complex constant 1+0i
    V.tensor_scalar(Zre[:, 0, :], wre[:], 0.0, 1.0, op.mult, op.add)
    V.tensor_scalar(Zim[:, 0, :], wre[:], 0.0, 0.0, op.mult, op.add)

    # interleaved power spine + doubling bulk: GW level j follows zp[j], Z32
    # level j follows za[j], keeping each level's inputs hot
    zp = []
    z0 = new_zpair("z0")
    csq_parts(z0[0][:], z0[1][:], wre[:], wim[:])
    zp.append(z0)
    cdouble(GWre, GWim, zp[0][0], zp[0][1], 1, conj_stored=True)
    for j in range(1, 5):                     # z^2, z^4, z^8, z^16
        zj = new_zpair(f"z{1 << j}")
        csq_parts(zj[0][:], zj[1][:], zp[-1][0][:], zp[-1][1][:])
        zp.append(zj)
        if j < 4:
            cdouble(GWre, GWim, zp[j][0], zp[j][1], 1 << j, conj_stored=True)
    za = []
    z32t = new_zpair("z32")
    csq_parts(z32t[0][:], z32t[1][:], zp[4][0][:], zp[4][1][:])
    za.append(z32t)                           # z^32
    cdouble_seg(GWre, GWim, zp[4][0], zp[4][1], 0, 16, 8, conj_stored=True)
    cdouble_seg(GWre, GWim, zp[4][0], zp[4][1], 8, 24, 8, conj_stored=True)
    cdouble(Zre, Zim, za[0][0], za[0][1], 1, conj_stored=False)
    for j in range(1, 4):                     # z^64, z^128, z^256
        zj = new_zpair(f"za{j}")
        csq_parts(zj[0][:], zj[1][:], za[-1][0][:], za[-1][1][:])
        za.append(zj)
        cdouble(Zre, Zim, za[j][0], za[j][1], 1 << j, conj_stored=False)

    # ---------------- forward half-transforms: 6 chunk-groups up front so the
    # PE stays busy for the whole duration of the prologue chain
    for b in range(6):
        emit_G(b)

    # D_rep [128, 128] (D broadcast down partitions)
    ones = p_small.tile([1, 128], dt.float32, tag="ones")
    V.memset(ones[:], 1.0)
    ps_d = p_psk.tile([128, 512], dt.float32, tag="psk")
    T.matmul(ps_d[0:128, 0:HS], ones[:], dtile[:], start=True, stop=True)
    D_rep = p_drep.tile([128, 128], dt.float32, tag="drep")
    A.copy(D_rep[:], ps_d[0:128, 0:HS])

    # mode-sum: two contraction-64 matmuls per channel
    ks_all = p_ks.tile([32, 16, 128], dt.bfloat16, tag="ksall", bufs=1)
    for g in range(4):
        kp_g = p_psk.tile([32, 32, 16], dt.float32, tag="psk", name=f"kp{g}")
        for hl in range(32):
            h = 32 * g + hl
            T.matmul(kp_g[0:32, hl, :], GWre_t[:, :, h], Zre_t[:, :, h],
                     start=True, stop=False)
            T.matmul(kp_g[0:32, hl, :], GWim_t[:, :, h], Zim_t[:, :, h],
                     start=False, stop=True)
        A.copy(ks_all[:, :, 32 * g:32 * g + 32], kp_g[:].transpose([0, 2, 1]))

    # kc assembly: kc[l = 32a+b, h] with a = 4c+al; 4 partition-offset DMAs
    kc_big = p_kc.tile([128, NJ, 128], dt.bfloat16, tag="kc")
    for al in range(4):
        dst = kc_big[32 * al:32 * al + 32, :, :]
        srcv = ks_all[:].rearrange("b (c al) h -> b c al h", al=4)[:, :, al, :]
        eng = A if al % 2 == 0 else nc.sync
        eng.dma_start(dst, srcv)

    # ---------------- K_f via packed half-DFT (reuse forward stationaries, bf16 moving)
    pks = {}
    for t_ in range(2):
        pks[t_] = p_psk.tile([128, NFT, 128], dt.float32, tag="psk", name=f"kdft{t_}")
        for ft in range(NFT):
            for c in range(NJ):
                T.matmul(pks[t_][:, ft, :], ch_tile(c, t_, ft), kc_big[:, c, :],
                         start=(c == 0), stop=(c == NJ - 1))

    # ---------------- main loop
    def kb(t):
        return t[:].unsqueeze(1).broadcast_to([128, 4, 128])

    yft = {}
    xft = {}

    def emit_combine(b):
        # X = Gp +/- Gb (even/odd freq tiles). ft0/ft1 ride the PE as
        # +/-identity matmul accumulations (evacuated by ACT); ft2/ft3 on DVE.
        gb = gplanes[b]
        gp = gplanes.get(b - 1)
        xs_list = []
        for ft in range(NFT):
            gbc, gbs = gb[ft]
            gpc, gps = gp[ft] if gp is not None else (zg, zg)
            xc = p_x.tile([128, 512], dt.bfloat16, tag=f"x{ft}c", name=f"x{b}_{ft}c")
            xs = p_x.tile([128, 512], dt.bfloat16, tag=f"x{ft}s", name=f"x{b}_{ft}s")
            cop = op.add if ft < 2 else op.subtract
            TT(xc[:], gpc[:], gbc[:], cop)
            TT(xs[:], gps[:], gbs[:], cop)
            xs_list.append((xc, xs))
        xft[b] = xs_list

    emit_combine(0)
    emit_combine(1)

    # filter tiles (bf16): skip connection u*D folds in as K'_f = K_f + D.
    # ft0 tiles (incl. krD0) first: the ft-major pointwise/inverse can start
    # on them while the later tiles are still being built.
    krA, krBC = [], []
    for ft in range(NFT):
        ta = p_krep.tile([128, 128], dt.bfloat16, tag=f"krA{ft}")
        tb = p_krep.tile([128, 128], dt.bfloat16, tag=f"krB{ft}")
        TT(ta[:], pks[0][:, ft, :], D_rep[:], op.add)
        A.copy(tb[:], pks[1][:, ft, :])
        krA.append(ta)
        krBC.append(tb)
        if ft == 0:
            krD0 = p_krep.tile([128, 128], dt.bfloat16, tag="krD0")
            TT(krD0[:], pks[0][:, 0, :], D_rep[:], op.add)
            # row 0 of the D-tensor holds K_Nyquist + D
            TT(krD0[0:1, :], pks[1][:, 0, :][0:1, :], D_rep[0:1, :], op.add)
            V.memset(tb[0:1, :], 0.0)         # Ki slot for f=0/Nyquist is zero

    def emit_pointwise(b):
        yr_t, yi_t = [], []
        xs_list = xft.pop(b)
        for ft in range(NFT):
            xc, xs = xs_list[ft]
            xc3 = xc[:].rearrange("p (b h) -> p b h", b=4)
            xs3 = xs[:].rearrange("p (b h) -> p b h", b=4)
            dten = krD0 if ft == 0 else krA[ft]
            # Yr = Xc*Kr - Xs*Ki ; Yi = Xc*Ki + Xs*Kr(+D special row0)
            t1 = p_tmp.tile([128, 512], dt.bfloat16, tag="t1")
            t2 = p_tmp.tile([128, 512], dt.bfloat16, tag="t2")
            TT(t1[:].rearrange("p (b h) -> p b h", b=4), xc3, kb(krA[ft]), op.mult)
            GT(t2[:].rearrange("p (b h) -> p b h", b=4), xs3, kb(krBC[ft]), op.mult)
            yr = p_yf.tile([128, 512], dt.bfloat16, tag=f"yr{ft}", name=f"yr{b}_{ft}")
            TT(yr[:], t1[:], t2[:], op.subtract)
            t3 = p_tmp.tile([128, 512], dt.bfloat16, tag="t3")
            t4 = p_tmp.tile([128, 512], dt.bfloat16, tag="t4")
            eng3 = GT if ft < 3 else TT
            eng3(t3[:].rearrange("p (b h) -> p b h", b=4), xc3, kb(krBC[ft]), op.mult)
            TT(t4[:].rearrange("p (b h) -> p b h", b=4), xs3, kb(dten), op.mult)
            yi = p_yf.tile([128, 512], dt.bfloat16, tag=f"yi{ft}", name=f"yi{b}_{ft}")
            TT(yi[:], t3[:], t4[:], op.add)
            yr_t.append(yr)
            yi_t.append(yi)
        yft[b] = (yr_t, yi_t)

    def emit_inverse(b):
        # ft-major accumulation: the first matmuls only need the ft=0 pointwise
        # tiles, so the PE starts the inverse while later tiles are still
        # computing. The last block goes lt-major instead so each psum's
        # evacuation overlaps the remaining matmuls (shorter pipeline drain).
        yr_t, yi_t = yft.pop(b)
        if b == NBLK - 1:
            for lt in range(NLT):
                py = p_ps.tile([128, 512], dt.float32, tag="ps", name=f"py{b}_{lt}")
                for ft in range(NFT):
                    T.matmul(py[:], ai_tile(0, ft, lt), yr_t[ft][:],
                             start=(ft == 0), stop=False)
                    T.matmul(py[:], ai_tile(1, ft, lt), yi_t[ft][:],
                             start=False, stop=(ft == NFT - 1))
                c_out = 4 * b + lt
                yo = p_yout.tile([128, 512], dt.float32, tag="yout")
                A.copy(yo[:], py[:])
                A.dma_start(y_ap[:, 128 * c_out:128 * c_out + 128, :].transpose([1, 0, 2]),
                            yo[:].rearrange("p (b h) -> p b h", b=4))
            return
        pys = []
        for lt in range(NLT):
            pys.append(p_ps.tile([128, 512], dt.float32, tag="ps", name=f"py{b}_{lt}"))
        for ft in range(NFT):
            for lt in range(NLT):
                T.matmul(pys[lt][:], ai_tile(0, ft, lt), yr_t[ft][:],
                         start=(ft == 0), stop=False)
                T.matmul(pys[lt][:], ai_tile(1, ft, lt), yi_t[ft][:],
                         start=False, stop=(ft == NFT - 1))
        for lt in range(NLT):
            c_out = 4 * b + lt
            yo = p_yout.tile([128, 512], dt.float32, tag="yout")
            A.copy(yo[:], pys[lt][:])
            A.dma_start(y_ap[:, 128 * c_out:128 * c_out + 128, :].transpose([1, 0, 2]),
                        yo[:].rearrange("p (b h) -> p b h", b=4))

    emit_pointwise(0)
    for b in range(NBLK):
        if b + 1 < NBLK:
            emit_pointwise(b + 1)
        if b + 2 < NBLK:
            emit_combine(b + 2)
        if b + 6 < NBLK:
            emit_G(b + 6)
        emit_inverse(b)


def _build_program():
    if _prog.built:
        return
    nc = bacc.Bacc("TRN2", target_bir_lowering=False, debug=False,
                   num_devices=NCORES)
    aps = {}
    aps["u"] = nc.dram_tensor("u", [B, L, HS], dt.bfloat16, kind="ExternalInput").ap()
    aps["D"] = nc.dram_tensor("D", [1, HS], dt.float32, kind="ExternalInput").ap()
    aps["logdt"] = nc.dram_tensor("logdt", [1, HS], dt.float32, kind="ExternalInput").ap()
    aps["Wre"] = nc.dram_tensor("Wre", [N, HS], dt.float32, kind="ExternalInput").ap()
    aps["Wim"] = nc.dram_tensor("Wim", [N, HS], dt.float32, kind="ExternalInput").ap()
    aps["Lre"] = nc.dram_tensor("Lre", [1, N], dt.float32, kind="ExternalInput").ap()
    aps["Lim"] = nc.dram_tensor("Lim", [1, N], dt.float32, kind="ExternalInput").ap()
    aps["CH"] = nc.dram_tensor("CH", [2, NFT, NJ, 128, 128], dt.bfloat16,
                               kind="ExternalInput").ap()
    aps["AI"] = nc.dram_tensor("AI", [2, NFT, NLT, 128, 128], dt.bfloat16,
                               kind="ExternalInput").ap()
    aps["y"] = nc.dram_tensor("y", [B, L, HS], dt.float32, kind="ExternalOutput").ap()
    with tile.TileContext(nc, trace_sim=False) as tc:
        with ExitStack() as ctx:
            _emit_kernel(nc, tc, ctx, aps)
    nc.compile()
    _prog.nc = nc
    _prog.CH, _prog.AI = build_constants()
    _prog.built = True


def make_in_maps(u, D, log_dt, W_re, W_im, Lambda_re, Lambda_im):
    _build_program()
    in_maps = []
    for c in range(NCORES):
        h0 = c * HS
        in_maps.append({
            "u": np.ascontiguousarray(u[:, :, h0:h0 + HS]).astype(ml_dtypes.bfloat16),
            "D": np.ascontiguousarray(D[h0:h0 + HS], dtype=f32).reshape(1, HS),
            "logdt": np.ascontiguousarray(log_dt[h0:h0 + HS], dtype=f32).reshape(1, HS),
            "Wre": np.ascontiguousarray(W_re[h0:h0 + HS].T, dtype=f32),
            "Wim": np.ascontiguousarray(W_im[h0:h0 + HS].T, dtype=f32),
            "Lre": np.ascontiguousarray(Lambda_re, dtype=f32).reshape(1, N),
            "Lim": np.ascontiguousarray(Lambda_im, dtype=f32).reshape(1, N),
            "CH": _prog.CH,
            "AI": _prog.AI,
        })
    return in_maps


LAST_RESULTS = None


def kernel(u, D, Lambda_re, Lambda_im, log_dt, W_re, W_im):
    global LAST_RESULTS
    from concourse.bass_utils import run_bass_kernel_spmd
    in_maps = make_in_maps(u, D, log_dt, W_re, W_im, Lambda_re, Lambda_im)
    res = run_bass_kernel_spmd(_prog.nc, in_maps, core_ids=list(range(NCORES)))
    LAST_RESULTS = res
    y = np.concatenate([res.results[c]["y"] for c in range(NCORES)], axis=2)
    return y.astype(np.float32)


# revision 49
# speedup vs baseline: 1.0133x; 1.0133x over previous
"""DSS (Diagonal State Space) layer as a Bass/Tile kernel for 8 Trainium2 NeuronCores.

Per core (channels H sharded 8 x 128), overlap-save FFT convolution with
chunk-reused half-transforms:
  1. DSS-exp kernel k[l,h] = Re(sum_n W[h,n] z^l), z = exp(dt_h*Lambda_n), built
     on-device: transcendentals via fused scalar_tensor_tensor Horner chains
     (ACT LUTs are too coarse for the z^511 power compounding), then complex
     doubling chains build W z^b (b<32, fp32) and z^(32a) (a<16) planes, and a
     per-channel pair of contraction-64 PE matmuls does the mode-sum.
  2. K_f = half-rfft of k via PE matmuls reusing the forward DFT stationaries
     (bf16 moving side: full PE rate at 128-wide free dim); skip connection
     u*D folds into the filter as K' = K + D.
  3. Overlap-save, F=1024, hop 512: block b's spectrum is X_b = G_{b-1} +
     (-1)^f G_b, where G_c is the half-transform of 512-sample chunk c,
     computed ONCE per chunk -- halving forward matmul work. Frequencies are
     reordered even/odd on the host so the (-1)^f combine is a plain per-tile
     add (even tiles) / subtract (odd tiles) in bf16 on DVE (2x rate).
  4. Pointwise complex multiply in bf16 split across DVE and gpsimd; inverse
     rfft as PE matmuls (bf16 moving, ft-major accumulation so it starts after
     the first frequency tile's pointwise).

Pipeline: u input (bf16) streams on the SP queue; six chunk-group forward
transforms are emitted up front so the PE covers the whole prologue chain;
combines run one block ahead and pointwise one block ahead of the inverse;
evacuations and y output ride the ACT queue. All engine ops keep partition
base 0 and matched fp32-family operand dtypes (BIR verifier requirements).
"""

import sys

for _p in ("/opt/trn_rl_repo", "/opt/trn_rl_repo/concourse"):
    if _p not in sys.path:
        sys.path.insert(0, _p)

import numpy as np
import ml_dtypes
from contextlib import ExitStack

import concourse.bacc as bacc
import concourse.tile as tile
import concourse.mybir as mybir

dt = mybir.dt
f32 = np.float32

B, L, H, N = 4, 4096, 1024, 64
LK = 512
F = 1024          # FFT length (overlap-save)
HOP = 512         # block hop = new samples per block
NCORES = 8
HS = H // NCORES  # 128 channels per core
NBLK = L // HOP   # 8
NFT = 4           # freq tiles of 128 (perm order: ft0,1 = even freqs, ft2,3 = odd)
NJ = 4            # contraction chunks (128 each) per 512-sample half-transform
NLT = 4           # output l-tiles per block


# ---------------------------------------------------------------- host constants
def build_constants():
    evens = np.arange(0, 512, 2)
    odds = np.arange(1, 512, 2)
    perm = np.concatenate([evens, odds])        # packed-freq reordering

    l5 = np.arange(512, dtype=np.float64)[:, None]
    fr = perm[None, :].astype(np.float64)
    ang = 2 * np.pi * l5 * fr / F
    C = np.cos(ang)                             # [512 l, 512 perm-freq]
    S = -np.sin(ang)
    S[:, 0] = (-1.0) ** np.arange(512)          # Nyquist row packed into sin slot 0

    CH = np.zeros((2, NFT, NJ, 128, 128))
    for t_ in range(2):
        M = C if t_ == 0 else S
        for ft in range(NFT):
            for j in range(NJ):
                CH[t_, ft, j] = M[128 * j:128 * j + 128, 128 * ft:128 * ft + 128]

    lc = 512 + np.arange(512, dtype=np.float64)[None, :]   # valid circular outputs
    frc = perm[:, None].astype(np.float64)
    cf_ = np.where(frc == 0, 1.0, 2.0)
    Ar = cf_ * np.cos(2 * np.pi * frc * lc / F) / F
    Ai = -(2.0 / F) * np.sin(2 * np.pi * frc * lc / F)
    Ai[0, :] = ((-1.0) ** lc[0]) / F                        # Nyquist inverse row
    AI = np.zeros((2, NFT, NLT, 128, 128))
    for ft in range(NFT):
        for lt in range(NLT):
            AI[0, ft, lt] = Ar[128 * ft:128 * ft + 128, 128 * lt:128 * lt + 128]
            AI[1, ft, lt] = Ai[128 * ft:128 * ft + 128, 128 * lt:128 * lt + 128]
    return CH.astype(ml_dtypes.bfloat16), AI.astype(ml_dtypes.bfloat16)


# Horner coefficient lists (highest degree first)
def _fact(k):
    r = 1.0
    for i in range(2, k + 1):
        r *= i
    return r


EXP7 = [1.0 / _fact(k) for k in range(7, -1, -1)]            # e^x, |x| <~ 0.35
EXP8 = [1.0 / _fact(k) for k in range(8, -1, -1)]            # e^x, |x| <~ 1.0
SIN9 = [1.0 / _fact(9), -1.0 / _fact(7), 1.0 / _fact(5), -1.0 / _fact(3), 1.0]
COSC = [1.0 / _fact(10), -1.0 / _fact(8), 1.0 / _fact(6), -1.0 / _fact(4),
        1.0 / _fact(2)]


class _Prog:
    def __init__(self):
        self.nc = None
        self.built = False


_prog = _Prog()


def _emit_kernel(nc, tc, ctx, aps):
    V = nc.vector
    A = nc.scalar
    T = nc.tensor
    u_ap = aps["u"]; y_ap = aps["y"]
    ch_ap = aps["CH"]; ai_ap = aps["AI"]
    TT = V.tensor_tensor
    GT = nc.gpsimd.tensor_tensor
    op = mybir.AluOpType

    # ---------------- pools
    p_ch = ctx.enter_context(tc.tile_pool(name="ch", bufs=1))
    p_ai = ctx.enter_context(tc.tile_pool(name="ai", bufs=1))
    p_uch = ctx.enter_context(tc.tile_pool(name="uch", bufs=10))
    p_g = ctx.enter_context(tc.tile_pool(name="g", bufs=6))
    p_x = ctx.enter_context(tc.tile_pool(name="x", bufs=2))
    p_tmp = ctx.enter_context(tc.tile_pool(name="tmp", bufs=3))
    p_yf = ctx.enter_context(tc.tile_pool(name="yf", bufs=2))
    p_yout = ctx.enter_context(tc.tile_pool(name="yout", bufs=2))
    p_krep = ctx.enter_context(tc.tile_pool(name="krep", bufs=1))
    p_kc = ctx.enter_context(tc.tile_pool(name="kc", bufs=1))
    p_ks = ctx.enter_context(tc.tile_pool(name="ks", bufs=4))
    p_gw = ctx.enter_context(tc.tile_pool(name="gw", bufs=1))
    p_zp = ctx.enter_context(tc.tile_pool(name="zp", bufs=9))
    p_small = ctx.enter_context(tc.tile_pool(name="small", bufs=1))
    p_gwtmp = ctx.enter_context(tc.tile_pool(name="gwtmp", bufs=1))
    p_drep = ctx.enter_context(tc.tile_pool(name="drep", bufs=1))
    p_ps = ctx.enter_context(tc.tile_pool(name="ps", bufs=6, space="PSUM"))
    p_psk = ctx.enter_context(tc.tile_pool(name="psk", bufs=2, space="PSUM"))

    # ---------------- small parameter tiles first (they gate the whole k prologue)
    logdt = p_small.tile([N, HS], dt.float32, tag="logdt")
    A.dma_start(logdt[:], aps["logdt"][:].broadcast_to([N, HS]))
    Lre_c = p_small.tile([N, 1], dt.float32, tag="lre")
    A.dma_start(Lre_c[:], aps["Lre"][:].transpose([1, 0]))
    Lim_c = p_small.tile([N, 1], dt.float32, tag="lim")
    A.dma_start(Lim_c[:], aps["Lim"][:].transpose([1, 0]))
    dtile = p_small.tile([1, HS], dt.float32, tag="dtile")
    A.dma_start(dtile[:], aps["D"][:])
    wret = p_small.tile([N, HS], dt.float32, tag="wret")
    A.dma_start(wret[:], aps["Wre"][:])
    wimt = p_small.tile([N, HS], dt.float32, tag="wimt")
    A.dma_start(wimt[:], aps["Wim"][:])

    # ---------------- stationaries: CH on SP queue (gates forward), AI on ACT queue
    ch_big = {}

    def load_ch(ft):
        for t_ in range(2):
            tl = p_ch.tile([128, NJ, 128], dt.bfloat16, tag=f"ch{t_}_{ft}",
                           name=f"chb{t_}_{ft}")
            eng = nc.sync if t_ == 0 else A
            eng.dma_start(tl[:], ch_ap[t_, ft].transpose([1, 0, 2]))
            ch_big[(t_, ft)] = tl

    load_ch(0)


    def ch_tile(j, t_, ft):
        return ch_big[(t_, ft)][:, j, :]

    def ai_tile(t_, ft, lt):
        return ai_big[(t_, ft)][:, lt, :]

    # u chunks: all on the SP queue (no cross-dependencies -> never head-of-line blocked)
    chunks = {}

    def get_chunk(c):
        if c not in chunks:
            t_u = p_uch.tile([128, 4, 128], dt.bfloat16, tag="uch", name=f"uch{c}")
            nc.sync.dma_start(t_u[:], u_ap[:, 128 * c:128 * c + 128, :].transpose([1, 0, 2]))
            chunks[c] = t_u
        return chunks[c]

    for c in range(4):
        get_chunk(c)
    for ft in range(1, NFT):
        load_ch(ft)
    for c in range(4, 12):
        get_chunk(c)

    ai_big = {}
    for t_ in range(2):
        for ft in range(NFT):
            tl = p_ai.tile([128, NLT, 128], dt.bfloat16, tag=f"ai{t_}_{ft}",
                           name=f"aib{t_}_{ft}")
            nc.sync.dma_start(tl[:], ai_ap[t_, ft].transpose([1, 0, 2]))
            ai_big[(t_, ft)] = tl

    # zero G planes for the block -1 "previous" spectrum
    zg = p_small.tile([128, 512], dt.bfloat16, tag="zg")
    V.memset(zg[:], 0.0)

    # ---------------- half-transform G of one 512-sample chunk-group
    gplanes = {}

    def emit_G(b):
        planes = []
        for ft in range(NFT):
            pc = p_ps.tile([128, 512], dt.float32, tag="ps", name=f"gpc{b}_{ft}")
            psn = p_ps.tile([128, 512], dt.float32, tag="ps", name=f"gps{b}_{ft}")
            for j in range(NJ):
                ch = get_chunk(4 * b + j)
                mv = ch[:].rearrange("p b h -> p (b h)")
                T.matmul(pc[:], ch_tile(j, 0, ft), mv, start=(j == 0), stop=(j == NJ - 1))
                T.matmul(psn[:], ch_tile(j, 1, ft), mv, start=(j == 0), stop=(j == NJ - 1))
            gc = p_g.tile([128, 512], dt.bfloat16, tag=f"g{ft}c", bufs=6, name=f"g{b}_{ft}c")
            A.copy(gc[:], pc[:])
            gs = p_g.tile([128, 512], dt.bfloat16, tag=f"g{ft}s", bufs=6, name=f"g{b}_{ft}s")
            A.copy(gs[:], psn[:])
            planes.append((gc, gs))
        gplanes[b] = planes

    # ---------------- prologue part 1: transcendentals + power chains.
    # All PE pieces here (outer products, D_rep) are emitted BEFORE the G
    # matmuls so the DVE chain is never gated on the forward-transform queue.
    STT = V.scalar_tensor_tensor
    GP = nc.gpsimd

    def horner(dst, x, coefs, eng):
        # P(x) = sum c_k x^k via fused steps s <- (s + c)*x, one op per coeff
        eng.tensor_scalar_mul(dst, x, float(coefs[0]))
        for c in coefs[1:-1]:
            eng.scalar_tensor_tensor(dst, dst, float(c), x, op.add, op.mult)
        eng.tensor_scalar_add(dst, dst, float(coefs[-1]))

    # dt = exp(logdt) = (exp(logdt/8))^8, computed on the 64-partition
    # broadcast copy so the n x h outer products become per-partition-scalar
    # DVE ops (no PE in the chain's critical path)
    x8 = p_small.tile([N, HS], dt.float32, tag="x8")
    V.tensor_scalar_mul(x8[:], logdt[:], 0.125)
    e8 = p_small.tile([N, HS], dt.float32, tag="e8")
    horner(e8[:], x8[:], EXP7, V)
    dtv = p_small.tile([N, HS], dt.float32, tag="dtv")
    t_a = p_small.tile([N, HS], dt.float32, tag="sq1")
    TT(t_a[:], e8[:], e8[:], op.mult)
    t_b = p_small.tile([N, HS], dt.float32, tag="sq2")
    TT(t_b[:], t_a[:], t_a[:], op.mult)
    TT(dtv[:], t_b[:], t_b[:], op.mult)

    # -exp(Lre) = -(exp(Lre/8))^8 on [N, 1] columns
    xl = p_small.tile([N, 1], dt.float32, tag="xl")
    V.tensor_scalar_mul(xl[:], Lre_c[:], 0.125)
    el8 = p_small.tile([N, 1], dt.float32, tag="el8")
    horner(el8[:], xl[:], EXP7, V)
    t_c = p_small.tile([N, 1], dt.float32, tag="sq3")
    TT(t_c[:], el8[:], el8[:], op.mult)
    t_d = p_small.tile([N, 1], dt.float32, tag="sq4")
    TT(t_d[:], t_c[:], t_c[:], op.mult)
    negel = p_small.tile([N, 1], dt.float32, tag="negel")
    t_e = p_small.tile([N, 1], dt.float32, tag="sq5")
    TT(t_e[:], t_d[:], t_d[:], op.mult)
    V.tensor_scalar_mul(negel[:], t_e[:], -1.0)

    # half-angle pieces: a[n,h] = -e^{Lre_n} dt_h ; b[n,h] = Lim_n dt_h
    ah = p_small.tile([N, HS], dt.float32, tag="ah")
    V.tensor_scalar(ah[:], dtv[:], negel[:], 0.5, op.mult, op.mult)
    bh = p_small.tile([N, HS], dt.float32, tag="bh")
    V.tensor_scalar(bh[:], dtv[:], Lim_c[:], 0.5, op.mult, op.mult)

    # trig: exp on DVE, cos-poly on gpsimd, sin-poly on DVE (concurrent streams)
    ea = p_small.tile([N, HS], dt.float32, tag="ea")
    horner(ea[:], ah[:], EXP8, V)
    ub = p_small.tile([N, HS], dt.float32, tag="ub")
    TT(ub[:], bh[:], bh[:], op.mult)
    cp = p_small.tile([N, HS], dt.float32, tag="cp")
    horner(cp[:], ub[:], COSC, V)
    cb = p_small.tile([N, HS], dt.float32, tag="cb")
    tq = p_small.tile([N, HS], dt.float32, tag="hornq", bufs=2)
    GT(tq[:], cp[:], ub[:], op.mult)
    V.tensor_scalar(cb[:], tq[:], -1.0, 1.0, op.mult, op.add)   # cos = 1 - u*P(u)
    sp = p_small.tile([N, HS], dt.float32, tag="sp")
    horner(sp[:], ub[:], SIN9, V)
    sb = p_small.tile([N, HS], dt.float32, tag="sb")
    TT(sb[:], sp[:], bh[:], op.mult)          # sin(b/2)

    wre = p_small.tile([N, HS], dt.float32, tag="wre")
    TT(wre[:], ea[:], cb[:], op.mult)
    wim = p_small.tile([N, HS], dt.float32, tag="wim")
    TT(wim[:], ea[:], sb[:], op.mult)

    def csq_parts(dre, dim_, sre, sim):
        t2 = p_small.tile([N, HS], dt.float32, tag="csq2", bufs=2)
        GT(t2[:], sim, sim, op.mult)
        t1 = p_small.tile([N, HS], dt.float32, tag="csq1", bufs=2)
        TT(t1[:], sre, sre, op.mult)
        TT(dre, t1[:], t2[:], op.subtract)
        STT(dim_, sre, 2.0, sim, op.mult, op.mult)   # 2*sre*sim fused

    def new_zpair(nm):
        zr = p_zp.tile([N, HS], dt.float32, tag="zp", name=f"{nm}r")
        zi = p_zp.tile([N, HS], dt.float32, tag="zp", name=f"{nm}i")
        return zr, zi

    # ---------------- GW planes [N, 32, HS] b-major: Re(W z^b) and -Im(W z^b)
    # in separate base-0 tiles; Z [N, 16, HS]: Re / +Im of z^(32a). All chain
    # ops stay partition-aligned at base 0; the mode-sum contracts re and im
    # with two accumulating matmuls per channel.
    GWre_t = p_gw.tile([N, 32, HS], dt.float32r, tag="gwre")
    GWim_t = p_gw.tile([N, 32, HS], dt.float32r, tag="gwim")
    Zre_t = p_gw.tile([N, 16, HS], dt.float32r, tag="zre")
    Zim_t = p_gw.tile([N, 16, HS], dt.float32r, tag="zim")
    GWre = GWre_t[:]
    GWim = GWim_t[:]
    Zre = Zre_t[:]
    Zim = Zim_t[:]
    V.tensor_scalar_mul(GWre[:, 0, :], wret[:], 1.0)
    V.tensor_scalar_mul(GWim[:, 0, :], wimt[:], -1.0)

    def cdouble_seg(pre, pim, zr, zi, s0, d0, w, conj_stored):
        # planes[d0:d0+w, :] = planes[s0:s0+w, :] * z (b-major); DVE gets
        # pre-mults + cross-re + add-re, gpsimd gets cross-im + add-im
        zre = zr[:].unsqueeze(1).broadcast_to([N, w, HS])
        zim = zi[:].unsqueeze(1).broadcast_to([N, w, HS])
        t2 = p_gwtmp.tile([N, 8, HS], dt.float32, tag="gt2", bufs=2)
        t4 = p_gwtmp.tile([N, 8, HS], dt.float32, tag="gt2", bufs=2)
        GT(t4[:, 0:w, :], pre[:, s0:s0 + w, :], zim, op.mult)
        TT(t2[:, 0:w, :], pim[:, s0:s0 + w, :], zim, op.mult)
        TT(pre[:, d0:d0 + w, :], pre[:, s0:s0 + w, :], zre, op.mult)
        TT(pim[:, d0:d0 + w, :], pim[:, s0:s0 + w, :], zre, op.mult)
        TT(pre[:, d0:d0 + w, :], pre[:, d0:d0 + w, :], t2[:, 0:w, :],
           op.add if conj_stored else op.subtract)
        GP.tensor_tensor(pim[:, d0:d0 + w, :], pim[:, d0:d0 + w, :], t4[:, 0:w, :],
                         op.subtract if conj_stored else op.add)

    def cdouble(pre, pim, zr, zi, w, conj_stored):
        cdouble_seg(pre, pim, zr, zi, 0, w, w, conj_stored)

    # a=0 plane of Z is the complex constant 1+0i
    V.tensor_scalar(Zre[:, 0, :], wre[:], 0.0, 1.0, op.mult, op.add)
    V.tensor_scalar(Zim[:, 0, :], wre[:], 0.0, 0.0, op.mult, op.add)

    # interleaved power spine + doubling bulk: GW level j follows zp[j], Z32
    # level j follows za[j], keeping each level's inputs hot
    zp = []
    z0 = new_zpair("z0")
    csq_parts(z0[0][:], z0[1][:], wre[:], wim[:])
    zp.append(z0)
    cdouble(GWre, GWim, zp[0][0], zp[0][1], 1, conj_stored=True)
    for j in range(1, 5):                     # z^2, z^4, z^8, z^16
        zj = new_zpair(f"z{1 << j}")
        csq_parts(zj[0][:], zj[1][:], zp[-1][0][:], zp[-1][1][:])
        zp.append(zj)
        if j < 4:
            cdouble(GWre, GWim, zp[j][0], zp[j][1], 1 << j, conj_stored=True)
    za = []
    z32t = new_zpair("z32")
    csq_parts(z32t[0][:], z32t[1][:], zp[4][0][:], zp[4][1][:])
    za.append(z32t)                           # z^32
    cdouble_seg(GWre, GWim, zp[4][0], zp[4][1], 0, 16, 8, conj_stored=True)
    cdouble_seg(GWre, GWim, zp[4][0], zp[4][1], 8, 24, 8, conj_stored=True)
    cdouble(Zre, Zim, za[0][0], za[0][1], 1, conj_stored=False)
    for j in range(1, 4):                     # z^64, z^128, z^256
        zj = new_zpair(f"za{j}")
        csq_parts(zj[0][:], zj[1][:], za[-1][0][:], za[-1][1][:])
        za.append(zj)
        cdouble(Zre, Zim, za[j][0], za[j][1], 1 << j, conj_stored=False)

    # ---------------- forward half-transforms: 6 chunk-groups up front so the
    # PE stays busy for the whole duration of the prologue chain
    for b in range(6):
        emit_G(b)

    # D_rep [128, 128] (D broadcast down partitions)
    ones = p_small.tile([1, 128], dt.float32, tag="ones")
    V.memset(ones[:], 1.0)
    ps_d = p_psk.tile([128, 512], dt.float32, tag="psk")
    T.matmul(ps_d[0:128, 0:HS], ones[:], dtile[:], start=True, stop=True)
    D_rep = p_drep.tile([128, 128], dt.float32, tag="drep")
    A.copy(D_rep[:], ps_d[0:128, 0:HS])

    # mode-sum: two contraction-64 matmuls per channel
    ks_all = p_ks.tile([32, 16, 128], dt.bfloat16, tag="ksall", bufs=1)
    for g in range(4):
        kp_g = p_psk.tile([32, 32, 16], dt.float32, tag="psk", name=f"kp{g}")
        for hl in range(32):
            h = 32 * g + hl
            T.matmul(kp_g[0:32, hl, :], GWre_t[:, :, h], Zre_t[:, :, h],
                     start=True, stop=False)
            T.matmul(kp_g[0:32, hl, :], GWim_t[:, :, h], Zim_t[:, :, h],
                     start=False, stop=True)
        A.copy(ks_all[:, :, 32 * g:32 * g + 32], kp_g[:].transpose([0, 2, 1]))

    # kc assembly: kc[l = 32a+b, h] with a = 4c+al; 4 partition-offset DMAs
    kc_big = p_kc.tile([128, NJ, 128], dt.bfloat16, tag="kc")
    for al in range(4):
        dst = kc_big[32 * al:32 * al + 32, :, :]
        srcv = ks_all[:].rearrange("b (c al) h -> b c al h", al=4)[:, :, al, :]
        eng = A if al % 2 == 0 else nc.sync
        eng.dma_start(dst, srcv)

    # ---------------- K_f via packed half-DFT (reuse forward stationaries, bf16 moving)
    pks = {}
    for t_ in range(2):
        pks[t_] = p_psk.tile([128, NFT, 128], dt.float32, tag="psk", name=f"kdft{t_}")
        for ft in range(NFT):
            for c in range(NJ):
                T.matmul(pks[t_][:, ft, :], ch_tile(c, t_, ft), kc_big[:, c, :],
                         start=(c == 0), stop=(c == NJ - 1))

    # ---------------- main loop
    def kb(t):
        return t[:].unsqueeze(1).broadcast_to([128, 4, 128])

    yft = {}
    xft = {}

    def emit_combine(b):
        # X = Gp +/- Gb (even/odd freq tiles). ft0/ft1 ride the PE as
        # +/-identity matmul accumulations (evacuated by ACT); ft2/ft3 on DVE.
        gb = gplanes[b]
        gp = gplanes.get(b - 1)
        xs_list = []
        for ft in range(NFT):
            gbc, gbs = gb[ft]
            gpc, gps = gp[ft] if gp is not None else (zg, zg)
            xc = p_x.tile([128, 512], dt.bfloat16, tag=f"x{ft}c", name=f"x{b}_{ft}c")
            xs = p_x.tile([128, 512], dt.bfloat16, tag=f"x{ft}s", name=f"x{b}_{ft}s")
            cop = op.add if ft < 2 else op.subtract
            TT(xc[:], gpc[:], gbc[:], cop)
            TT(xs[:], gps[:], gbs[:], cop)
            xs_list.append((xc, xs))
        xft[b] = xs_list

    emit_combine(0)
    emit_combine(1)

    # filter tiles (bf16): skip connection u*D folds in as K'_f = K_f + D.
    # ft0 tiles (incl. krD0) first: the ft-major pointwise/inverse can start
    # on them while the later tiles are still being built.
    krA, krBC = [], []
    for ft in range(NFT):
        ta = p_krep.tile([128, 128], dt.bfloat16, tag=f"krA{ft}")
        tb = p_krep.tile([128, 128], dt.bfloat16, tag=f"krB{ft}")
        TT(ta[:], pks[0][:, ft, :], D_rep[:], op.add)
        A.copy(tb[:], pks[1][:, ft, :])
        krA.append(ta)
        krBC.append(tb)
        if ft == 0:
            krD0 = p_krep.tile([128, 128], dt.bfloat16, tag="krD0")
            TT(krD0[:], pks[0][:, 0, :], D_rep[:], op.add)
            # row 0 of the D-tensor holds K_Nyquist + D
            TT(krD0[0:1, :], pks[1][:, 0, :][0:1, :], D_rep[0:1, :], op.add)
            V.memset(tb[0:1, :], 0.0)         # Ki slot for f=0/Nyquist is zero

    def emit_pointwise(b):
        yr_t, yi_t = [], []
        xs_list = xft.pop(b)
        for ft in range(NFT):
            xc, xs = xs_list[ft]
            xc3 = xc[:].rearrange("p (b h) -> p b h", b=4)
            xs3 = xs[:].rearrange("p (b h) -> p b h", b=4)
            dten = krD0 if ft == 0 else krA[ft]
            # Yr = Xc*Kr - Xs*Ki ; Yi = Xc*Ki + Xs*Kr(+D special row0)
            t1 = p_tmp.tile([128, 512], dt.bfloat16, tag="t1")
            t2 = p_tmp.tile([128, 512], dt.bfloat16, tag="t2")
            TT(t1[:].rearrange("p (b h) -> p b h", b=4), xc3, kb(krA[ft]), op.mult)
            GT(t2[:].rearrange("p (b h) -> p b h", b=4), xs3, kb(krBC[ft]), op.mult)
            yr = p_yf.tile([128, 512], dt.bfloat16, tag=f"yr{ft}", name=f"yr{b}_{ft}")
            TT(yr[:], t1[:], t2[:], op.subtract)
            t3 = p_tmp.tile([128, 512], dt.bfloat16, tag="t3")
            t4 = p_tmp.tile([128, 512], dt.bfloat16, tag="t4")
            eng3 = GT if ft < 2 else TT
            eng3(t3[:].rearrange("p (b h) -> p b h", b=4), xc3, kb(krBC[ft]), op.mult)
            TT(t4[:].rearrange("p (b h) -> p b h", b=4), xs3, kb(dten), op.mult)
            yi = p_yf.tile([128, 512], dt.bfloat16, tag=f"yi{ft}", name=f"yi{b}_{ft}")
            TT(yi[:], t3[:], t4[:], op.add)
            yr_t.append(yr)
            yi_t.append(yi)
        yft[b] = (yr_t, yi_t)

    def emit_inverse(b):
        # ft-major accumulation: the first matmuls only need the ft=0 pointwise
        # tiles, so the PE starts the inverse while later tiles are still
        # computing. The last block goes lt-major instead so each psum's
        # evacuation overlaps the remaining matmuls (shorter pipeline drain).
        yr_t, yi_t = yft.pop(b)
        if b == NBLK - 1:
            for lt in range(NLT):
                py = p_ps.tile([128, 512], dt.float32, tag="ps", name=f"py{b}_{lt}")
                for ft in range(NFT):
                    T.matmul(py[:], ai_tile(0, ft, lt), yr_t[ft][:],
                             start=(ft == 0), stop=False)
                    T.matmul(py[:], ai_tile(1, ft, lt), yi_t[ft][:],
                             start=False, stop=(ft == NFT - 1))
                c_out = 4 * b + lt
                yo = p_yout.tile([128, 512], dt.float32, tag="yout")
                A.copy(yo[:], py[:])
                A.dma_start(y_ap[:, 128 * c_out:128 * c_out + 128, :].transpose([1, 0, 2]),
                            yo[:].rearrange("p (b h) -> p b h", b=4))
            return
        pys = []
        for lt in range(NLT):
            pys.append(p_ps.tile([128, 512], dt.float32, tag="ps", name=f"py{b}_{lt}"))
        for ft in range(NFT):
            for lt in range(NLT):
                T.matmul(pys[lt][:], ai_tile(0, ft, lt), yr_t[ft][:],
                         start=(ft == 0), stop=False)
                T.matmul(pys[lt][:], ai_tile(1, ft, lt), yi_t[ft][:],
                         start=False, stop=(ft == NFT - 1))
        for lt in range(NLT):
            c_out = 4 * b + lt
            yo = p_yout.tile([128, 512], dt.float32, tag="yout")
            A.copy(yo[:], pys[lt][:])
            A.dma_start(y_ap[:, 128 * c_out:128 * c_out + 128, :].transpose([1, 0, 2]),
                        yo[:].rearrange("p (b h) -> p b h", b=4))

    emit_pointwise(0)
    for b in range(NBLK):
        if b + 1 < NBLK:
            emit_pointwise(b + 1)
        if b + 2 < NBLK:
            emit_combine(b + 2)
        if b + 6 < NBLK:
            emit_G(b + 6)
        emit_inverse(b)


def _build_program():
    if _prog.built:
        return
    nc = bacc.Bacc("TRN2", target_bir_lowering=False, debug=False,
                   num_devices=NCORES)
    aps = {}
    aps["u"] = nc.dram_tensor("u", [B, L, HS], dt.bfloat16, kind="ExternalInput").ap()
    aps["D"] = nc.dram_tensor("D", [1, HS], dt.float32, kind="ExternalInput").ap()
    aps["logdt"] = nc.dram_tensor("logdt", [1, HS], dt.float32, kind="ExternalInput").ap()
    aps["Wre"] = nc.dram_tensor("Wre", [N, HS], dt.float32, kind="ExternalInput").ap()
    aps["Wim"] = nc.dram_tensor("Wim", [N, HS], dt.float32, kind="ExternalInput").ap()
    aps["Lre"] = nc.dram_tensor("Lre", [1, N], dt.float32, kind="ExternalInput").ap()
    aps["Lim"] = nc.dram_tensor("Lim", [1, N], dt.float32, kind="ExternalInput").ap()
    aps["CH"] = nc.dram_tensor("CH", [2, NFT, NJ, 128, 128], dt.bfloat16,
                               kind="ExternalInput").ap()
    aps["AI"] = nc.dram_tensor("AI", [2, NFT, NLT, 128, 128], dt.bfloat16,
                               kind="ExternalInput").ap()
    aps["y"] = nc.dram_tensor("y", [B, L, HS], dt.float32, kind="ExternalOutput").ap()
    with tile.TileContext(nc, trace_sim=False) as tc:
        with ExitStack() as ctx:
            _emit_kernel(nc, tc, ctx, aps)
    nc.compile()
    _prog.nc = nc
    _prog.CH, _prog.AI = build_constants()
    _prog.built = True


def make_in_maps(u, D, log_dt, W_re, W_im, Lambda_re, Lambda_im):
    _build_program()
    in_maps = []
    for c in range(NCORES):
        h0 = c * HS
        in_maps.append({
            "u": np.ascontiguousarray(u[:, :, h0:h0 + HS]).astype(ml_dtypes.bfloat16),
            "D": np.ascontiguousarray(D[h0:h0 + HS], dtype=f32).reshape(1, HS),
            "logdt": np.ascontiguousarray(log_dt[h0:h0 + HS], dtype=f32).reshape(1, HS),
            "Wre": np.ascontiguousarray(W_re[h0:h0 + HS].T, dtype=f32),
            "Wim": np.ascontiguousarray(W_im[h0:h0 + HS].T, dtype=f32),
            "Lre": np.ascontiguousarray(Lambda_re, dtype=f32).reshape(1, N),
            "Lim": np.ascontiguousarray(Lambda_im, dtype=f32).reshape(1, N),
            "CH": _prog.CH,
            "AI": _prog.AI,
        })
    return in_maps


LAST_RESULTS = None


def kernel(u, D, Lambda_re, Lambda_im, log_dt, W_re, W_im):
    global LAST_RESULTS
    from concourse.bass_utils import run_bass_kernel_spmd
    in_maps = make_in_maps(u, D, log_dt, W_re, W_im, Lambda_re, Lambda_im)
    res = run_bass_kernel_spmd(_prog.nc, in_maps, core_ids=list(range(NCORES)))
    LAST_RESULTS = res
    y = np.concatenate([res.results[c]["y"] for c in range(NCORES)], axis=2)
    return y.astype(np.float32)
